# revision 1
# baseline (speedup 1.0000x reference)
"""Trainium2 Bass kernel for EnhancedCondConv2d (moe_routing).

Data-parallel over batch: 8 cores x 2 samples each. Full inputs in,
full outputs back.

v3 pipeline (per core, samples software-pipelined):
  prologue(b): host-padded x DMA in 4 contiguous chunks (big DMA
               packets) + per-chunk DVE avgpool partials -> routing
               MLP -> rweights -> wgen from resident expert table
  conv(b):     3x3 grouped conv as 9 PSUM-accumulated shifted bf16
               matmuls (2 live banks / 8-row groups); ACT eviction to
               bf16 osb + f32 channel-sum accumulators
  post(b):     SE MLP -> cw; ACT in-place SE scale of osb; CBAM stats
               via PE matmuls against host const [I|1] (129th col =
               channel sum) + DVE channel max; 7x7 spatial conv as 14
               host-precomputed banded-Toeplitz bf16 matmuls ->
               sigmoid -> sw; final out = osb*sw + x in bf16 with
               residual read from SBUF, stores on two DMA queues.
Issue order: prologue(b+1) before post(b) so sample b+1's x/routing/
wgen overlap sample b's conv and post phases.
"""

import math
from contextlib import ExitStack

import numpy as np

import concourse.bass as bass
import concourse.bacc as bacc
import concourse.mybir as mybir
import concourse.tile as tile
from concourse.bass_utils import run_bass_kernel_spmd

F32 = mybir.dt.float32
BF16 = mybir.dt.bfloat16
FP8 = mybir.dt.float8e4
AX = mybir.AxisListType
ALU = mybir.AluOpType
ACTF = mybir.ActivationFunctionType

B, CI, CO, H, W, E, KK, RR = 16, 128, 128, 128, 128, 16, 3, 8
NCORES = 8
BL = B // NCORES  # 2 samples per core
EPS = 1e-5
HW = H * W
IKK = CI * KK * KK  # 1152
BNS = 1.0 / math.sqrt(1.0 + EPS)
HP, WP = H + 2, W + 2  # host-padded

_CACHE = {}


def _build_module():
    nc = bacc.Bacc("TRN2", target_bir_lowering=False, debug=False)

    xp_d = nc.dram_tensor("x2p", [BL, CI, HP, WP], BF16, kind="ExternalInput").ap()
    ew_d = nc.dram_tensor("experts_w", [128, 16, IKK], FP8, kind="ExternalInput").ap()
    idc_d = nc.dram_tensor("idc", [128, 129], BF16, kind="ExternalInput").ap()
    mc_d = nc.dram_tensor("mc", [128, 14, 128], BF16, kind="ExternalInput").ap()
    rw1t_d = nc.dram_tensor("rw1t", [CI, 16], F32, kind="ExternalInput").ap()
    rw2t_d = nc.dram_tensor("rw2t", [16, CI], F32, kind="ExternalInput").ap()
    rw3t_d = nc.dram_tensor("rw3t", [CI, 16], F32, kind="ExternalInput").ap()
    caw1t_d = nc.dram_tensor("caw1t", [CO, 16], F32, kind="ExternalInput").ap()
    caw2t_d = nc.dram_tensor("caw2t", [16, CO], F32, kind="ExternalInput").ap()
    g1_d = nc.dram_tensor("rbn1_g", [16], F32, kind="ExternalInput").ap()
    b1_d = nc.dram_tensor("rbn1_b", [16], F32, kind="ExternalInput").ap()
    g2_d = nc.dram_tensor("rbn2_g", [CI], F32, kind="ExternalInput").ap()
    b2_d = nc.dram_tensor("rbn2_b", [CI], F32, kind="ExternalInput").ap()
    rb3_d = nc.dram_tensor("rb3", [E], F32, kind="ExternalInput").ap()
    cag1_d = nc.dram_tensor("ca_bn1_g", [16], F32, kind="ExternalInput").ap()
    cab1_d = nc.dram_tensor("ca_bn1_b", [16], F32, kind="ExternalInput").ap()
    cag2_d = nc.dram_tensor("ca_bn2_g", [CO], F32, kind="ExternalInput").ap()
    cab2_d = nc.dram_tensor("ca_bn2_b", [CO], F32, kind="ExternalInput").ap()
    sag_d = nc.dram_tensor("sa_bn_g", [1], F32, kind="ExternalInput").ap()
    sab_d = nc.dram_tensor("sa_bn_b", [1], F32, kind="ExternalInput").ap()
    bmask_d = nc.dram_tensor("bmask", [128, 8], FP8, kind="ExternalInput").ap()
    e16t_d = nc.dram_tensor("e16t", [16, 128], BF16, kind="ExternalInput").ap()

    out_d = nc.dram_tensor("out", [BL, CO, H, W], BF16, kind="ExternalOutput").ap()

    srw_d = nc.dram_tensor("scr_rw", [BL, E], F32).ap()
    ssw_d = nc.dram_tensor("scr_sw", [BL, H, W], BF16).ap()

    with tile.TileContext(nc) as tc, ExitStack() as ctx:
        _kernel_body(
            ctx, tc,
            xp_d, ew_d, idc_d, mc_d, rw1t_d, rw2t_d, rw3t_d, caw1t_d, caw2t_d,
            g1_d, b1_d, g2_d, b2_d, rb3_d, cag1_d, cab1_d, cag2_d, cab2_d,
            sag_d, sab_d, bmask_d, e16t_d, out_d, srw_d, ssw_d,
        )
    nc.compile()
    return nc


def _kernel_body(ctx, tc,
                 xp_d, ew_d, idc_d, mc_d, rw1t_d, rw2t_d, rw3t_d, caw1t_d,
                 caw2t_d, g1_d, b1_d, g2_d, b2_d, rb3_d, cag1_d, cab1_d,
                 cag2_d, cab2_d, sag_d, sab_d, bmask_d, e16t_d, out_d,
                 srw_d, ssw_d):
    nc = tc.nc

    cpool = ctx.enter_context(tc.tile_pool(name="const", bufs=1))
    xpool = ctx.enter_context(tc.tile_pool(name="xp", bufs=2))
    opool = ctx.enter_context(tc.tile_pool(name="op", bufs=2))
    wpool = ctx.enter_context(tc.tile_pool(name="wp", bufs=2))
    spool = ctx.enter_context(tc.tile_pool(name="sp", bufs=2))
    fpool = ctx.enter_context(tc.tile_pool(name="fp", bufs=3))
    x8pool = ctx.enter_context(tc.tile_pool(name="x8p", bufs=2))

    pconv = ctx.enter_context(tc.tile_pool(name="pc", bufs=3, space="PSUM"))
    pw = ctx.enter_context(tc.tile_pool(name="pw", bufs=2, space="PSUM"))
    ptp = ctx.enter_context(tc.tile_pool(name="ptp", bufs=2, space="PSUM"))

    # ---------- constants (small queues: gpsimd/scalar) ----------
    ecr = cpool.tile([128, 16, IKK], FP8, tag="ecr")
    eck = ecr.rearrange("p o (i k) -> p o k i", k=9)

    def load_ecr():
        # issued after sample 0's x chunks so x0 gets full DMA bandwidth
        for u in range(4):
            deng = nc.sync if u % 2 == 0 else nc.scalar
            deng.dma_start(ecr[:, 4 * u:4 * u + 4, :],
                           ew_d[:, 4 * u:4 * u + 4, :])

    idc = cpool.tile([128, 129], BF16, tag="idc")
    nc.gpsimd.dma_start(idc, idc_d)
    mc = cpool.tile([128, 14, 128], BF16, tag="mc")
    nc.gpsimd.dma_start(mc, mc_d)

    rw1t = cpool.tile([CI, 16], F32, tag="rw1t")
    nc.gpsimd.dma_start(rw1t, rw1t_d)
    rw2t = cpool.tile([16, CI], F32, tag="rw2t")
    nc.gpsimd.dma_start(rw2t, rw2t_d)
    rw3t = cpool.tile([CI, 16], F32, tag="rw3t")
    nc.gpsimd.dma_start(rw3t, rw3t_d)
    caw1t = cpool.tile([CO, 16], F32, tag="caw1t")
    nc.gpsimd.dma_start(caw1t, caw1t_d)
    caw2t = cpool.tile([16, CO], F32, tag="caw2t")
    nc.gpsimd.dma_start(caw2t, caw2t_d)

    def vec_const(dst_tag, src_ap, n, scale):
        raw = cpool.tile([n, 1], F32, tag=dst_tag + "_r")
        nc.gpsimd.dma_start(raw, src_ap.unsqueeze(1))
        out = cpool.tile([n, 1], F32, tag=dst_tag)
        nc.vector.tensor_scalar_mul(out, raw, float(scale))
        return out

    gs1 = vec_const("gs1", g1_d, 16, BNS / HW)
    bb1 = vec_const("bb1", b1_d, 16, 1.0)
    gs2 = vec_const("gs2", g2_d, CI, BNS)
    bb2 = vec_const("bb2", b2_d, CI, 1.0)
    gsca1 = vec_const("gsca1", cag1_d, 16, BNS / HW / 8.0)
    bbca1 = vec_const("bbca1", cab1_d, 16, 1.0)
    gsca2 = vec_const("gsca2", cag2_d, CO, BNS)
    bbca2 = vec_const("bbca2", cab2_d, CO, 1.0)

    rb3r = cpool.tile([1, E], F32, tag="rb3r")
    nc.gpsimd.dma_start(rb3r, rb3_d.unsqueeze(0))

    gssa = cpool.tile([128, 1], F32, tag="gssa")
    nc.gpsimd.dma_start(gssa, sag_d.unsqueeze(0).partition_broadcast(128))
    nc.vector.tensor_scalar_mul(gssa, gssa, BNS)
    bssa = cpool.tile([128, 1], F32, tag="bssa")
    nc.gpsimd.dma_start(bssa, sab_d.unsqueeze(0).partition_broadcast(128))
    bmask = cpool.tile([128, 8], FP8, tag="bmask")
    nc.gpsimd.dma_start(bmask, bmask_d)
    e16t = cpool.tile([16, 128], BF16, tag="e16t")
    nc.gpsimd.dma_start(e16t, e16t_d)

    # x row chunks (host-padded: contiguous on both sides)
    XCH = [(0, 33), (33, 65), (65, 97), (97, 130)]

    def prologue(b):
        xp = xpool.tile([128, HP, WP], BF16, tag="x_pad")
        x8 = x8pool.tile([128, HP, WP], FP8, tag="x8")
        for u, (r0, r1) in enumerate(XCH):
            deng = nc.sync if u % 2 == 0 else nc.scalar
            deng.dma_start(xp[:, r0:r1, :], xp_d[b, :, r0:r1, :])
        if b == 0:
            load_ecr()

        # fused per-chunk cast to fp8 + avgpool partial on the ACT engine
        # (pads are zero, safe to include in the sum)
        osb = opool.tile([128, H, W], BF16, tag="out_sb")
        pAB = spool.tile([128, 4], F32, tag="pAB")
        for u, (r0, r1) in enumerate(XCH):
            nc.scalar.activation(
                x8[:, r0:r1, :], xp[:, r0:r1, :], ACTF.Copy,
                accum_out=pAB[:, u:u + 1])
        psum_t = spool.tile([128, 1], F32, tag="psum_t")
        nc.vector.tensor_reduce(psum_t, pAB, AX.X, ALU.add)

        # routing MLP
        mm1 = ptp.tile([16, 1], F32, tag="r", bufs=1)
        nc.tensor.matmul(mm1, rw1t, psum_t, start=True, stop=True)
        h1 = spool.tile([16, 1], F32, tag="h1")
        nc.scalar.activation(h1, mm1, ACTF.Relu, bias=bb1, scale=gs1)
        mm2 = ptp.tile([128, 1], F32, tag="r", bufs=1)
        nc.tensor.matmul(mm2, rw2t, h1, start=True, stop=True)
        gg = spool.tile([128, 1], F32, tag="gg")
        nc.scalar.activation(gg, mm2, ACTF.Sigmoid, bias=bb2, scale=gs2)
        mm3 = ptp.tile([1, E], F32, tag="r", bufs=1)
        nc.tensor.matmul(mm3, gg, rw3t, start=True, stop=True)
        lg = spool.tile([1, E], F32, tag="lg")
        nc.vector.tensor_add(lg, mm3, rb3r)
        mx = spool.tile([1, 1], F32, tag="mx")
        nc.vector.tensor_reduce(mx, lg, AX.X, ALU.max)
        mxn = spool.tile([1, 1], F32, tag="mxn")
        nc.gpsimd.tensor_scalar_mul(mxn, mx, -1.0)
        e16 = spool.tile([1, E], F32, tag="e16")
        nc.scalar.activation(e16, lg, ACTF.Exp, bias=mxn, scale=1.0)
        s1 = spool.tile([1, 1], F32, tag="s1")
        nc.vector.tensor_reduce(s1, e16, AX.X, ALU.add)
        rinv = spool.tile([1, 1], F32, tag="rinv")
        nc.vector.reciprocal(rinv, s1)
        rwrow = spool.tile([1, E], BF16, tag="rwrow")
        nc.gpsimd.tensor_scalar_mul(rwrow, e16, rinv)
        # broadcast rweights across partitions via the PE (no DRAM trip):
        # transpose [1,16]->[16,1], widen, then E16 selector matmul
        rwtp = ptp.tile([16, 1], BF16, tag="r", bufs=1)
        nc.tensor.matmul(rwtp, rwrow, idc[0:1, 0:1], is_transpose=True)
        rwt8 = spool.tile([16, 8], BF16, tag="rwt8", bufs=1)
        nc.vector.tensor_copy(rwt8, rwtp.broadcast_to([16, 8]))
        rwbp = ptp.tile([128, 8], F32, tag="r", bufs=1)
        nc.tensor.matmul(rwbp, e16t, rwt8, start=True, stop=True)
        rwblk = spool.tile([128, 8], FP8, tag="rwblk")
        nc.vector.tensor_tensor(rwblk, bmask, rwbp, ALU.mult)

        # wgen: w[i, k, o] = sum_e rw[e] experts[e, o, i, k]; fp8 out
        # (experts pre-scaled x16 on host, x2 here -> w stored as w*32)
        wsb = wpool.tile([128, 10, CO], FP8, tag="wsb")
        nc.vector.memset(wsb[:, 9, :], 0.0)
        for kt in range(3):
            pwt = pw.tile([128, 384], F32, tag="pw", name=f"pw{b}_{kt}")
            for og in range(16):
                for j in range(3):
                    k = 3 * kt + j
                    dst = pwt[:, j * 128 + og * 8:j * 128 + og * 8 + 8]
                    nc.tensor.matmul(dst, eck[:, og, k, :], rwblk,
                                     start=True, stop=True)
            for j in range(3):
                nc.vector.tensor_scalar_mul(
                    wsb[:, 3 * kt + j, :], pwt[:, j * 128:(j + 1) * 128], 2.0)

        # --- analytic channel sums of the conv output (linearity):
        # cps[o] = sum_k sum_i w[k,i,o] * S_k[i], with S_k the 128x128
        # window sum of padded x at tap k. Pads are zero, so S_k derives
        # from the pool total T minus 2 row sums / 2 column sums.
        # Gives cw BEFORE the conv so the SE scale folds into eviction.
        ed = spool.tile([128, 4], F32, tag="ed", bufs=1)
        nc.vector.tensor_reduce(ed[:, 0:1], xp[:, 1, :], AX.X, ALU.add)
        nc.vector.tensor_reduce(ed[:, 1:2], xp[:, 128, :], AX.X, ALU.add)
        nc.vector.tensor_reduce(ed[:, 2:3], xp[:, :, 1], AX.X, ALU.add)
        nc.vector.tensor_reduce(ed[:, 3:4], xp[:, :, 128], AX.X, ALU.add)
        # A_kh = T - (kh==0)*row128sum - (kh==2)*row1sum
        A3 = spool.tile([128, 3], F32, tag="A3", bufs=1)
        nc.vector.tensor_tensor(A3[:, 0:1], psum_t, ed[:, 1:2], ALU.subtract)
        nc.vector.tensor_copy(A3[:, 1:2], psum_t)
        nc.vector.tensor_tensor(A3[:, 2:3], psum_t, ed[:, 0:1], ALU.subtract)
        # column strips: D_c[kh] = colsum_c - excluded edge elements
        D1 = spool.tile([128, 3], F32, tag="D1", bufs=1)
        nc.vector.tensor_tensor(D1[:, 0:1], ed[:, 2:3],
                                xp[:, 128, 1].unsqueeze(1), ALU.subtract)
        nc.vector.tensor_copy(D1[:, 1:2], ed[:, 2:3])
        nc.vector.tensor_tensor(D1[:, 2:3], ed[:, 2:3],
                                xp[:, 1, 1].unsqueeze(1), ALU.subtract)
        D128 = spool.tile([128, 3], F32, tag="D128", bufs=1)
        nc.vector.tensor_tensor(D128[:, 0:1], ed[:, 3:4],
                                xp[:, 128, 128].unsqueeze(1), ALU.subtract)
        nc.vector.tensor_copy(D128[:, 1:2], ed[:, 3:4])
        nc.vector.tensor_tensor(D128[:, 2:3], ed[:, 3:4],
                                xp[:, 1, 128].unsqueeze(1), ALU.subtract)
        S9 = spool.tile([128, 3, 3], F32, tag="S9", bufs=1)  # [p, kw, kh]
        nc.vector.tensor_tensor(S9[:, 0, :], A3, D128, ALU.subtract)
        nc.vector.tensor_copy(S9[:, 1, :], A3)
        nc.vector.tensor_tensor(S9[:, 2, :], A3, D1, ALU.subtract)
        s8 = spool.tile([128, 3, 3], FP8, tag="s8", bufs=1)
        nc.vector.tensor_scalar_mul(s8, S9, 0.25)
        pcp = ptp.tile([128, 1], F32, tag="r", bufs=1)
        for k in range(9):
            kh, kw = k // 3, k % 3
            nc.tensor.matmul(pcp, wsb[:, k, :], s8[:, kw, kh].unsqueeze(1),
                             start=(k == 0), stop=(k == 8))
        cpsb = spool.tile([128, 1], F32, tag="cpsb", bufs=1)
        nc.vector.tensor_copy(cpsb, pcp)
        se1 = ptp.tile([16, 1], F32, tag="r", bufs=1)
        nc.tensor.matmul(se1, caw1t, cpsb, start=True, stop=True)
        ch = spool.tile([16, 1], F32, tag="ch")
        nc.scalar.activation(ch, se1, ACTF.Relu, bias=bbca1, scale=gsca1)
        se2 = ptp.tile([128, 1], F32, tag="r", bufs=1)
        nc.tensor.matmul(se2, caw2t, ch, start=True, stop=True)
        cw = spool.tile([128, 1], F32, tag="cw")
        nc.scalar.activation(cw, se2, ACTF.Sigmoid, bias=bbca2, scale=gsca2)
        cws = spool.tile([128, 1], F32, tag="cws")
        nc.vector.tensor_scalar_mul(cws, cw, 1.0 / 32.0)
        return xp, osb, wsb, x8, cws

    # DoubleRow tap pairs; the pair-dim stride must be EVEN (odd strides
    # crash the exec unit), so pair taps with matching column parity:
    # (k0, k1, rhs delta); tap 9 is the zero pad
    DRP = [(0, 3, WP), (1, 4, WP), (2, 5, WP), (6, 8, 2), (7, 9, -2)]

    def conv(b, st):
        xp, osb, wsb, x8, cws = st
        for hs in range(16):
            pcs = [pconv.tile([128, 512], F32, tag="c", name=f"pc{b}_{hs}_{i}")
                   for i in range(2)]
            for t, (k0, k1, delta) in enumerate(DRP):
                kh0, kw0 = k0 // 3, k0 % 3
                lhs = bass.AP(wsb.tensor, wsb.offset + k0 * CO,
                              [list(wsb.ap[0]), [(k1 - k0) * CO, 2], [1, CO]])
                for g in range(2):
                    r0 = hs * 8 + g * 4
                    eloff = (r0 + kh0) * WP + kw0
                    rhs = bass.AP(x8.tensor, x8.offset + eloff,
                                  [list(x8.ap[0]), [delta, 2], [WP, 4], [1, W]])
                    nc.tensor.matmul(pcs[g], lhs, rhs,
                                     start=(t == 0), stop=(t == 4),
                                     perf_mode=mybir.MatmulPerfMode.DoubleRow)
            for g in range(2):
                hr = hs * 8 + g * 4
                nc.scalar.activation(
                    osb[:, hr:hr + 4, :],
                    pcs[g].rearrange("p (a b) -> p a b", a=4),
                    ACTF.Copy, scale=cws)
        return None

    def post(b, st, cparts):
        xp, osb, wsb, x8, cws = st
        # CBAM stats: per-row matmul vs [I|1] + DVE channel max
        spmax = spool.tile([128, 134], BF16, tag="spmax")
        spsum = spool.tile([128, 134], BF16, tag="spsum")
        nc.vector.memset(spmax[:, 0:3], 0.0)
        nc.vector.memset(spmax[:, 131:134], 0.0)
        nc.vector.memset(spsum[:, 0:3], 0.0)
        nc.vector.memset(spsum[:, 131:134], 0.0)
        qi = 0
        for g in range(16):
            for h0, nr in ((8 * g, 3), (8 * g + 3, 3), (8 * g + 6, 2)):
                ptt = ptp.tile([128, 3, 129], F32, tag="ptt", name=f"ptt{b}_{qi}")
                for j in range(nr):
                    nc.tensor.matmul(ptt[:, j, :], osb[:, h0 + j, :], idc,
                                     start=True, stop=True)
                # evict to bf16 on ACT (keeps DVE for max reduces)
                spt = fpool.tile([128, 3, 129], BF16, tag="spt")
                nc.scalar.activation(spt[:, 0:nr, :], ptt[:, 0:nr, :],
                                     ACTF.Copy)
                nc.vector.tensor_reduce(
                    spmax[:, 3 + h0:3 + h0 + nr], spt[:, 0:nr, 0:128],
                    AX.X, ALU.max)
                nc.vector.tensor_copy(
                    spsum[:, 3 + h0:3 + h0 + nr], spt[:, 0:nr, 128])
                qi += 1

        # CBAM 7x7 conv: 14 banded matmuls (host-precomputed Toeplitz)
        pswt = pw.tile([128, 384], F32, tag="pw", name=f"psw{b}")
        psw = pswt[:, 0:128]
        for t in range(14):
            c, dh = t // 7, t % 7
            src = spsum if c == 0 else spmax
            nc.tensor.matmul(psw, mc[:, t, :], src[:, dh:dh + 128],
                             start=(t == 0), stop=(t == 13))
        swT = spool.tile([128, 128], BF16, tag="swT")
        nc.scalar.activation(swT, psw, ACTF.Sigmoid, bias=bssa, scale=gssa)
        pswh = pw.tile([128, 128], BF16, tag="pw", name=f"pswh{b}")
        nc.tensor.matmul(pswh, swT, idc[:, 0:128], is_transpose=True)
        swH = spool.tile([128, 128], BF16, tag="swH")
        nc.vector.tensor_copy(swH, pswh)
        nc.gpsimd.dma_start(ssw_d[b], swH)

        # final: out = (osb*cw)*sw + x
        for g in range(16):
            swbc = fpool.tile([128, 8, 128], BF16, tag="swbc")
            nc.gpsimd.dma_start(
                swbc, ssw_d[b, 8 * g:8 * g + 8, :].partition_broadcast(128))
            nc.vector.tensor_tensor(swbc, osb[:, 8 * g:8 * g + 8, :], swbc,
                                    ALU.mult)
            fo = fpool.tile([128, 8, 128], BF16, tag="fo")
            nc.vector.tensor_tensor(fo, swbc,
                                    xp[:, 1 + 8 * g:9 + 8 * g, 1:W + 1], ALU.add)
            nc.sync.dma_start(out_d[b, :, 8 * g:8 * g + 8, :], fo)

    # software pipeline: prologue(b+1) is issued before post(b)
    st0 = prologue(0)
    cp0 = conv(0, st0)
    st1 = prologue(1)
    post(0, st0, cp0)
    cp1 = conv(1, st1)
    post(1, st1, cp1)


def _host_prep(inp):
    import ml_dtypes
    experts = np.ascontiguousarray(inp["experts"], dtype=np.float32)
    # [E, O, I, K, K] -> [(o_sub, e)=128, og=16, IKK]
    ew = experts.reshape(E, 16, 8, IKK).transpose(2, 0, 1, 3)
    ew = np.ascontiguousarray(ew).reshape(128, 16, IKK)

    idc = np.zeros((128, 129), dtype=ml_dtypes.bfloat16)
    idc[np.arange(128), np.arange(128)] = 1.0
    idc[:, 128] = 1.0

    # banded Toeplitz matrices M[t=(c,dh)][w', w] = tap[c,dh,dw] at
    # w == w' + 3 - dw  (mean channel c=0 scaled by 1/CO)
    saw = np.asarray(inp["sa_w"], np.float32).reshape(2, 7, 7)
    mcm = np.zeros((14, 128, 128), dtype=np.float32)
    for t in range(14):
        c, dh = t // 7, t % 7
        for dw in range(7):
            val = float(saw[c, dh, dw]) * (1.0 / CO if c == 0 else 1.0)
            wp = np.arange(128)
            w = wp + 3 - dw
            m = (w >= 0) & (w < 128)
            mcm[t, wp[m], w[m]] += val
    mc = np.ascontiguousarray(mcm.transpose(1, 0, 2)).astype(ml_dtypes.bfloat16)

    e16t = np.zeros((16, 8, 16), dtype=ml_dtypes.bfloat16)
    for e in range(16):
        e16t[e, :, e] = 1.0
    e16t = e16t.reshape(16, 128)

    bm = np.zeros((8, 16, 8), dtype=ml_dtypes.float8_e4m3fn)
    for j in range(8):
        bm[j, :, j] = 1.0
    bm = bm.reshape(128, 8)

    shared = {
        "experts_w": (ew * 16.0).astype(ml_dtypes.float8_e4m3fn),
        "idc": idc,
        "mc": mc,
        "rw1t": np.ascontiguousarray(inp["rw1"].T, dtype=np.float32),
        "rw2t": np.ascontiguousarray(inp["rw2"].T, dtype=np.float32),
        "rw3t": np.ascontiguousarray(inp["rw3"].T, dtype=np.float32),
        "caw1t": np.ascontiguousarray(inp["ca_w1"].T, dtype=np.float32),
        "caw2t": np.ascontiguousarray(inp["ca_w2"].T, dtype=np.float32),
        "rbn1_g": np.asarray(inp["rbn1_g"], np.float32),
        "rbn1_b": np.asarray(inp["rbn1_b"], np.float32),
        "rbn2_g": np.asarray(inp["rbn2_g"], np.float32),
        "rbn2_b": np.asarray(inp["rbn2_b"], np.float32),
        "rb3": np.asarray(inp["rb3"], np.float32),
        "ca_bn1_g": np.asarray(inp["ca_bn1_g"], np.float32),
        "ca_bn1_b": np.asarray(inp["ca_bn1_b"], np.float32),
        "ca_bn2_g": np.asarray(inp["ca_bn2_g"], np.float32),
        "ca_bn2_b": np.asarray(inp["ca_bn2_b"], np.float32),
        "sa_bn_g": np.asarray(inp["sa_bn_g"], np.float32),
        "sa_bn_b": np.asarray(inp["sa_bn_b"], np.float32),
        "bmask": bm,
        "e16t": e16t,
    }
    x = np.asarray(inp["x"], np.float32)
    xpad = np.zeros((B, CI, HP, WP), dtype=ml_dtypes.bfloat16)
    xpad[:, :, 1:H + 1, 1:W + 1] = x.astype(ml_dtypes.bfloat16)
    in_maps = []
    for c in range(NCORES):
        m = dict(shared)
        m["x2p"] = np.ascontiguousarray(xpad[BL * c:BL * (c + 1)])
        in_maps.append(m)
    return in_maps


def get_module():
    if "nc" not in _CACHE:
        _CACHE["nc"] = _build_module()
    return _CACHE["nc"]


def kernel(**inputs):
    nc = get_module()
    in_maps = _host_prep(inputs)
    res = run_bass_kernel_spmd(nc, in_maps, core_ids=list(range(NCORES)))
    out = np.concatenate([r["out"] for r in res.results], axis=0)
    return out.astype(np.float32)



# revision 2
# speedup vs baseline: 1.1959x; 1.1959x over previous
"""Trainium2 Bass kernel for EnhancedCondConv2d (moe_routing).

Data-parallel over batch: 8 cores x 2 samples each. Full inputs in,
full outputs back.

v4 design (per core; routing batched across both samples):
  host:     x pre-cast to padded fp8 (conv input), x pre-transposed to
            [w, h, i] bf16 (residual input), exact f32 channel pool
            sums + 3x3 window sums, experts re-laid-out (osub,e)-major
            with contiguous i so LDWEIGHTS hits fast-weight-load.
  routing:  both samples' MLP+softmax chains run batched at t=0 from
            the host pool vector; wgen for both samples in one pass of
            144 FD=16 matmuls against the resident expert table.
  conv(b):  3x3 grouped conv as 4 DoubleRow pairs + 1 plain fp8 matmul
            per 4-row group; ACT eviction (x cw from the analytic SE
            path) to bf16 osb.
  post(b):  per-pixel channel stats via PE transpose matmuls against
            [I|1]; evicted (2-bank, 6-row chunks) into a TRANSPOSED
            activation tile osbT[w, h, o|sum|max]; DVE max-reduce;
            7x7 spatial conv as 14 banded-Toeplitz matmuls reading the
            sum/max columns; sigmoid -> swT[w, h]; final
            out_T = osbT * swT (free-dim broadcast, no partition
            broadcast needed) + xT, stored transposed; host untransposes.
"""

import math
from contextlib import ExitStack

import numpy as np

import concourse.bass as bass
import concourse.bacc as bacc
import concourse.mybir as mybir
import concourse.tile as tile
from concourse.bass_utils import run_bass_kernel_spmd

F32 = mybir.dt.float32
BF16 = mybir.dt.bfloat16
FP8 = mybir.dt.float8e4
AX = mybir.AxisListType
ALU = mybir.AluOpType
ACTF = mybir.ActivationFunctionType
DR = mybir.MatmulPerfMode.DoubleRow

B, CI, CO, H, W, E, KK, RR = 16, 128, 128, 128, 128, 16, 3, 8
NCORES = 8
BL = B // NCORES  # 2 samples per core
EPS = 1e-5
HW = H * W
BNS = 1.0 / math.sqrt(1.0 + EPS)
HP, WP = H + 2, W + 2  # host-padded
HT = H + 6  # h dim of osbT with +-3 padding for the 7x7 conv
OTW = CO + 2  # osbT row width: 128 o + sum + max

_CACHE = {}


def _build_module():
    nc = bacc.Bacc("TRN2", target_bir_lowering=False, debug=False)

    x8_d = nc.dram_tensor("x8", [BL, CI, HP, WP], FP8, kind="ExternalInput").ap()
    xt_d = nc.dram_tensor("xt", [BL, W, H, CI], BF16, kind="ExternalInput").ap()
    ew_d = nc.dram_tensor("ew", [128, 16, KK * KK, CI], FP8, kind="ExternalInput").ap()
    idc_d = nc.dram_tensor("idc", [128, 129], BF16, kind="ExternalInput").ap()
    mc_d = nc.dram_tensor("mc", [128, 14, 128], BF16, kind="ExternalInput").ap()
    pv_d = nc.dram_tensor("pv", [CI, BL], F32, kind="ExternalInput").ap()
    s8_d = nc.dram_tensor("s8", [CI, BL, 9], FP8, kind="ExternalInput").ap()
    rw1t_d = nc.dram_tensor("rw1t", [CI, 16], F32, kind="ExternalInput").ap()
    rw2t_d = nc.dram_tensor("rw2t", [16, CI], F32, kind="ExternalInput").ap()
    rw3t_d = nc.dram_tensor("rw3t", [CI, 16], F32, kind="ExternalInput").ap()
    caw1t_d = nc.dram_tensor("caw1t", [CO, 16], F32, kind="ExternalInput").ap()
    caw2t_d = nc.dram_tensor("caw2t", [16, CO], F32, kind="ExternalInput").ap()
    g1_d = nc.dram_tensor("rbn1_g", [16], F32, kind="ExternalInput").ap()
    b1_d = nc.dram_tensor("rbn1_b", [16], F32, kind="ExternalInput").ap()
    g2_d = nc.dram_tensor("rbn2_g", [CI], F32, kind="ExternalInput").ap()
    b2_d = nc.dram_tensor("rbn2_b", [CI], F32, kind="ExternalInput").ap()
    rb3_d = nc.dram_tensor("rb3", [E], F32, kind="ExternalInput").ap()
    cag1_d = nc.dram_tensor("ca_bn1_g", [16], F32, kind="ExternalInput").ap()
    cab1_d = nc.dram_tensor("ca_bn1_b", [16], F32, kind="ExternalInput").ap()
    cag2_d = nc.dram_tensor("ca_bn2_g", [CO], F32, kind="ExternalInput").ap()
    cab2_d = nc.dram_tensor("ca_bn2_b", [CO], F32, kind="ExternalInput").ap()
    sag_d = nc.dram_tensor("sa_bn_g", [1], F32, kind="ExternalInput").ap()
    sab_d = nc.dram_tensor("sa_bn_b", [1], F32, kind="ExternalInput").ap()
    bmask_d = nc.dram_tensor("bmask2", [128, 16], FP8, kind="ExternalInput").ap()
    e16t_d = nc.dram_tensor("e16t", [16, 128], BF16, kind="ExternalInput").ap()

    out_d = nc.dram_tensor("out", [BL, W, H, CO], BF16, kind="ExternalOutput").ap()

    with tile.TileContext(nc) as tc, ExitStack() as ctx:
        _kernel_body(
            ctx, tc,
            x8_d, xt_d, ew_d, idc_d, mc_d, pv_d, s8_d,
            rw1t_d, rw2t_d, rw3t_d, caw1t_d, caw2t_d,
            g1_d, b1_d, g2_d, b2_d, rb3_d, cag1_d, cab1_d, cag2_d, cab2_d,
            sag_d, sab_d, bmask_d, e16t_d, out_d,
        )
    nc.compile()
    return nc


def _kernel_body(ctx, tc,
                 x8_d, xt_d, ew_d, idc_d, mc_d, pv_d, s8_d,
                 rw1t_d, rw2t_d, rw3t_d, caw1t_d, caw2t_d,
                 g1_d, b1_d, g2_d, b2_d, rb3_d, cag1_d, cab1_d, cag2_d,
                 cab2_d, sag_d, sab_d, bmask_d, e16t_d, out_d):
    nc = tc.nc

    cpool = ctx.enter_context(tc.tile_pool(name="const", bufs=1))
    xpool = ctx.enter_context(tc.tile_pool(name="xp", bufs=2))
    opool = ctx.enter_context(tc.tile_pool(name="op", bufs=1))
    tpool = ctx.enter_context(tc.tile_pool(name="tp", bufs=2))
    wpool = ctx.enter_context(tc.tile_pool(name="wp", bufs=1))
    spool = ctx.enter_context(tc.tile_pool(name="sp", bufs=2))
    fpool = ctx.enter_context(tc.tile_pool(name="fp", bufs=4))

    pconv = ctx.enter_context(tc.tile_pool(name="pc", bufs=3, space="PSUM"))
    pbig = ctx.enter_context(tc.tile_pool(name="pb", bufs=2, space="PSUM"))
    psml = ctx.enter_context(tc.tile_pool(name="ps", bufs=1, space="PSUM"))

    # ---------- x8 loads first (sync queue) so conv(0) starts early ----
    XCH = [(0, 33), (33, 65), (65, 97), (97, 130)]
    x8s = []
    for b in range(BL):
        x8 = xpool.tile([128, HP, WP], FP8, tag="x8", name=f"x8_{b}")
        for u, (r0, r1) in enumerate(XCH):
            nc.sync.dma_start(x8[:, r0:r1, :], x8_d[b, :, r0:r1, :])
        x8s.append(x8)

    # experts on scalar queue (needed by wgen at ~5us)
    ecr = cpool.tile([128, 16, 9, CI], FP8, tag="ecr")
    for u in range(4):
        nc.scalar.dma_start(ecr[:, 4 * u:4 * u + 4], ew_d[:, 4 * u:4 * u + 4])

    # ---------- small constants (gpsimd queue) ----------
    idc = cpool.tile([128, 129], BF16, tag="idc")
    nc.gpsimd.dma_start(idc, idc_d)
    mc = cpool.tile([128, 14, 128], BF16, tag="mc")
    nc.gpsimd.dma_start(mc, mc_d)
    pv = cpool.tile([CI, BL], F32, tag="pv")
    nc.gpsimd.dma_start(pv, pv_d)
    s8 = cpool.tile([CI, BL, 9], FP8, tag="s8")
    nc.gpsimd.dma_start(s8, s8_d)

    rw1t = cpool.tile([CI, 16], F32, tag="rw1t")
    nc.gpsimd.dma_start(rw1t, rw1t_d)
    rw2t = cpool.tile([16, CI], F32, tag="rw2t")
    nc.gpsimd.dma_start(rw2t, rw2t_d)
    rw3t = cpool.tile([CI, 16], F32, tag="rw3t")
    nc.gpsimd.dma_start(rw3t, rw3t_d)
    caw1t = cpool.tile([CO, 16], F32, tag="caw1t")
    nc.gpsimd.dma_start(caw1t, caw1t_d)
    caw2t = cpool.tile([16, CO], F32, tag="caw2t")
    nc.gpsimd.dma_start(caw2t, caw2t_d)

    def vec_const(dst_tag, src_ap, n, scale):
        raw = cpool.tile([n, 1], F32, tag=dst_tag + "_r")
        nc.gpsimd.dma_start(raw, src_ap.unsqueeze(1))
        out = cpool.tile([n, 1], F32, tag=dst_tag)
        nc.vector.tensor_scalar_mul(out, raw, float(scale))
        return out

    gs1 = vec_const("gs1", g1_d, 16, BNS / HW)
    bb1 = vec_const("bb1", b1_d, 16, 1.0)
    gs2 = vec_const("gs2", g2_d, CI, BNS)
    bb2 = vec_const("bb2", b2_d, CI, 1.0)
    gsca1 = vec_const("gsca1", cag1_d, 16, BNS / HW / 8.0)
    bbca1 = vec_const("bbca1", cab1_d, 16, 1.0)
    gsca2 = vec_const("gsca2", cag2_d, CO, BNS)
    bbca2 = vec_const("bbca2", cab2_d, CO, 1.0)

    rb3r = cpool.tile([BL, E], F32, tag="rb3r")
    nc.gpsimd.dma_start(rb3r, rb3_d.unsqueeze(0).partition_broadcast(BL))

    gssa = cpool.tile([128, 1], F32, tag="gssa")
    nc.gpsimd.dma_start(gssa, sag_d.unsqueeze(0).partition_broadcast(128))
    nc.vector.tensor_scalar_mul(gssa, gssa, BNS)
    bssa = cpool.tile([128, 1], F32, tag="bssa")
    nc.gpsimd.dma_start(bssa, sab_d.unsqueeze(0).partition_broadcast(128))
    bmask = cpool.tile([128, 16], FP8, tag="bmask")
    nc.gpsimd.dma_start(bmask, bmask_d)
    e16t = cpool.tile([16, 128], BF16, tag="e16t")
    nc.gpsimd.dma_start(e16t, e16t_d)

    # ---------- routing (both samples batched) ----------
    def routing():
        mm1 = psml.tile([16, BL], F32, tag="r", bufs=1)
        nc.tensor.matmul(mm1, rw1t, pv, start=True, stop=True)
        h1 = spool.tile([16, BL], F32, tag="h1")
        nc.scalar.activation(h1, mm1, ACTF.Relu, bias=bb1, scale=gs1)
        mm2 = psml.tile([128, BL], F32, tag="r", bufs=1)
        nc.tensor.matmul(mm2, rw2t, h1, start=True, stop=True)
        gg = spool.tile([128, BL], F32, tag="gg")
        nc.scalar.activation(gg, mm2, ACTF.Sigmoid, bias=bb2, scale=gs2)
        mm3 = psml.tile([BL, E], F32, tag="r", bufs=1)
        nc.tensor.matmul(mm3, gg, rw3t, start=True, stop=True)
        lg = spool.tile([BL, E], F32, tag="lg")
        nc.vector.tensor_add(lg, mm3, rb3r)
        mx = spool.tile([BL, 1], F32, tag="mx")
        nc.vector.tensor_reduce(mx, lg, AX.X, ALU.max)
        mxn = spool.tile([BL, 1], F32, tag="mxn")
        nc.gpsimd.tensor_scalar_mul(mxn, mx, -1.0)
        e16 = spool.tile([BL, E], F32, tag="e16")
        nc.scalar.activation(e16, lg, ACTF.Exp, bias=mxn, scale=1.0)
        s1 = spool.tile([BL, 1], F32, tag="s1")
        nc.vector.tensor_reduce(s1, e16, AX.X, ALU.add)
        rinv = spool.tile([BL, 1], F32, tag="rinv")
        nc.vector.reciprocal(rinv, s1)
        rwrow = spool.tile([BL, E], BF16, tag="rwrow")
        nc.gpsimd.tensor_scalar_mul(rwrow, e16, rinv)
        # transpose [BL,16] -> [16,BL] on the PE, widen to 16 (j,b) cols
        rwtp = psml.tile([16, BL], BF16, tag="r", bufs=1)
        nc.tensor.matmul(rwtp, rwrow, idc[0:BL, 0:BL], is_transpose=True)
        rwt16 = spool.tile([16, 8, BL], BF16, tag="rwt16", bufs=1)
        nc.vector.tensor_copy(rwt16, rwtp.unsqueeze(1).broadcast_to([16, 8, BL]))
        rwbp = psml.tile([128, 16], F32, tag="r", bufs=1)
        nc.tensor.matmul(rwbp, e16t, rwt16, start=True, stop=True)
        rwblk = spool.tile([128, 16], FP8, tag="rwblk", bufs=1)
        nc.vector.tensor_tensor(rwblk, bmask, rwbp, ALU.mult)
        return rwblk

    # ---------- wgen (both samples): w[i, k, o] stored as w*32, fp8 ----
    def wgen(rwblk):
        wsbs = [wpool.tile([128, 9, CO], FP8, tag=f"wsb{b}", name=f"wsb{b}")
                for b in range(BL)]
        for kt in range(5):  # taps (2kt, 2kt+1); kt=4 -> tap 8 only
            ntap = 1 if kt == 4 else 2
            pwt = pbig.tile([128, 2, 512], F32, tag="big", name=f"pw{kt}")
            for j in range(ntap):
                k = 2 * kt + j
                for og in range(16):
                    nc.tensor.matmul(pwt[:, j, og * 16:og * 16 + 16],
                                     ecr[:, og, k, :], rwblk,
                                     start=True, stop=True)
                for b in range(BL):
                    src = bass.AP(pwt.tensor, pwt.offset + j * 512 + b,
                                  [list(pwt.ap[0]), [16, 16], [2, 8]])
                    nc.vector.tensor_scalar_mul(wsbs[b][:, k, :], src, 2.0)
        return wsbs

    # ---------- SE chain for sample b (analytic channel sums) ----------
    def se_chain(b, wsb):
        pcp = psml.tile([128, 1], F32, tag="r", bufs=1, name=f"pcp{b}")
        for k in range(9):
            nc.tensor.matmul(pcp, wsb[:, k, :], s8[:, b, k].unsqueeze(1),
                             start=(k == 0), stop=(k == 8))
        cpsb = spool.tile([128, 1], F32, tag="cpsb")
        nc.vector.tensor_copy(cpsb, pcp)
        se1 = psml.tile([16, 1], F32, tag="r", bufs=1, name=f"se1_{b}")
        nc.tensor.matmul(se1, caw1t, cpsb, start=True, stop=True)
        ch = spool.tile([16, 1], F32, tag="ch")
        nc.scalar.activation(ch, se1, ACTF.Relu, bias=bbca1, scale=gsca1)
        se2 = psml.tile([128, 1], F32, tag="r", bufs=1, name=f"se2_{b}")
        nc.tensor.matmul(se2, caw2t, ch, start=True, stop=True)
        cw = spool.tile([128, 1], F32, tag="cw")
        nc.scalar.activation(cw, se2, ACTF.Sigmoid, bias=bbca2, scale=gsca2)
        cws = spool.tile([128, 1], F32, tag="cws", name=f"cws{b}")
        nc.vector.tensor_scalar_mul(cws, cw, 1.0 / 32.0)
        return cws

    # conv tap pairs for DoubleRow (kw parity must match; rhs delta even)
    # (k0, k1) with rhs delta (k1-k0 decomposed); tap 7 runs plain.
    PAIRS = [(0, 3), (1, 4), (2, 5), (6, 8)]

    def conv(b, wsb, cws):
        x8 = x8s[b]
        osb = opool.tile([128, H, W], BF16, tag="osb", name=f"osb{b}")
        for hs in range(32):  # 4-row groups
            r0 = hs * 4
            pc = pconv.tile([128, 512], F32, tag="c", name=f"pc{b}_{hs}")
            for t, (k0, k1) in enumerate(PAIRS):
                kh0, kw0 = k0 // 3, k0 % 3
                kh1, kw1 = k1 // 3, k1 % 3
                delta = (kh1 - kh0) * WP + (kw1 - kw0)
                lhs = bass.AP(wsb.tensor, wsb.offset + k0 * CO,
                              [list(wsb.ap[0]), [(k1 - k0) * CO, 2], [1, CO]])
                eloff = (r0 + kh0) * WP + kw0
                rhs = bass.AP(x8.tensor, x8.offset + eloff,
                              [list(x8.ap[0]), [delta, 2], [WP, 4], [1, W]])
                nc.tensor.matmul(pc, lhs, rhs, start=(t == 0), stop=False,
                                 perf_mode=DR)
            # tap 7 (kh=2, kw=1) plain fp8 matmul (FWL path)
            rhs7 = bass.AP(x8.tensor, x8.offset + (r0 + 2) * WP + 1,
                           [list(x8.ap[0]), [WP, 4], [1, W]])
            nc.tensor.matmul(pc, wsb[:, 7, :], rhs7, start=False, stop=True)
            nc.scalar.activation(
                osb[:, r0:r0 + 4, :],
                pc.rearrange("p (a b) -> p a b", a=4),
                ACTF.Copy, scale=cws)
        return osb

    # ---------- transposed stats + spatial attention + final ----------
    # osbT[w, h(+3 pad each side), 0:128=o | 128=sum | 129=max]
    CH6 = [(6 * c, 6) for c in range(21)] + [(126, 2)]

    def post(b, osb, swT_out):
        osbT = tpool.tile([128, HT, OTW], BF16, tag="osbT", name=f"osbT{b}")
        # zero the sum/max columns of the h-pad rows
        nc.vector.memset(osbT[:, 0:3, CO:CO + 2], 0.0)
        nc.vector.memset(osbT[:, H + 3:H + 6, CO:CO + 2], 0.0)
        for h0, nr in CH6:
            ptt = pbig.tile([128, 2, 512], F32, tag="big", name=f"ptt{b}_{h0}")
            for j in range(nr):
                nc.tensor.matmul(ptt[:, j // 3, (j % 3) * 129:(j % 3) * 129 + 129],
                                 osb[:, h0 + j, :], idc, start=True, stop=True)
            # evict transposed rows (o + sum col) in one ACT instr
            nb = (nr + 2) // 3
            src = bass.AP(ptt.tensor, ptt.offset,
                          [list(ptt.ap[0]), [512, nb], [129, min(nr, 3)],
                           [1, 129]])
            dst = bass.AP(osbT.tensor, osbT.offset + (3 + h0) * OTW,
                          [list(osbT.ap[0]), [3 * OTW, nb], [OTW, min(nr, 3)],
                           [1, 129]])
            nc.scalar.activation(dst, src, ACTF.Copy)
            # channel max -> col 129
            mdst = bass.AP(osbT.tensor, osbT.offset + (3 + h0) * OTW + CO + 1,
                           [list(osbT.ap[0]), [OTW, nr]])
            nc.vector.tensor_reduce(mdst, osbT[:, 3 + h0:3 + h0 + nr, 0:CO],
                                    AX.X, ALU.max)

        # 7x7 spatial conv: 14 banded Toeplitz matmuls
        psw = pbig.tile([128, 2, 512], F32, tag="big", name=f"psw{b}")
        for t in range(14):
            c, dh = t // 7, t % 7
            col = CO + (0 if c == 0 else 1)
            src = bass.AP(osbT.tensor, osbT.offset + dh * OTW + col,
                          [list(osbT.ap[0]), [OTW, 128]])
            nc.tensor.matmul(psw[:, 0, 0:128], mc[:, t, :], src,
                             start=(t == 0), stop=(t == 13))
        swT = spool.tile([128, 128], BF16, tag="swT", name=f"swT{b}")
        nc.scalar.activation(swT, psw[:, 0, 0:128], ACTF.Sigmoid,
                             bias=bssa, scale=gssa)
        swT_out.append(swT)
        return osbT

    def final(b, osbT, swT):
        for g in range(16):
            h0 = 8 * g
            xtt = fpool.tile([128, 8, CI], BF16, tag="xtt", name=f"xt{b}_{g}")
            nc.scalar.dma_start(xtt, xt_d[b, :, h0:h0 + 8, :])
            fo = fpool.tile([128, 8, CO], BF16, tag="fo", name=f"fo{b}_{g}")
            nc.vector.tensor_tensor(
                fo, osbT[:, 3 + h0:3 + h0 + 8, 0:CO],
                swT[:, h0:h0 + 8].unsqueeze(2).broadcast_to([128, 8, CO]),
                ALU.mult)
            nc.vector.tensor_tensor(fo, fo, xtt, ALU.add)
            deng = nc.sync if g % 2 == 0 else nc.gpsimd
            deng.dma_start(out_d[b, :, h0:h0 + 8, :], fo)

    # ---------- schedule ----------
    rwblk = routing()
    wsbs = wgen(rwblk)
    cws0 = se_chain(0, wsbs[0])
    cws1 = se_chain(1, wsbs[1])
    swTs = []
    osb0 = conv(0, wsbs[0], cws0)
    osbT0 = post(0, osb0, swTs)
    osb1 = conv(1, wsbs[1], cws1)
    final(0, osbT0, swTs[0])
    osbT1 = post(1, osb1, swTs)
    final(1, osbT1, swTs[1])


def _host_prep(inp):
    import ml_dtypes
    x = np.asarray(inp["x"], np.float32)

    # padded fp8 conv input
    x8 = np.zeros((B, CI, HP, WP), dtype=ml_dtypes.float8_e4m3fn)
    x8[:, :, 1:H + 1, 1:W + 1] = x.astype(ml_dtypes.float8_e4m3fn)
    # transposed bf16 residual input [b, w, h, i]
    xt = np.ascontiguousarray(x.transpose(0, 3, 2, 1)).astype(ml_dtypes.bfloat16)

    # exact channel pool sums [i, b]
    xs = x.sum(axis=(2, 3))  # [B, I]
    # 3x3 window sums of zero-padded x: S[kh, kw] over the 128x128 window
    # starting at padded (kh, kw). Derived from total minus edge strips.
    x8f = np.zeros((B, CI, HP, WP), np.float32)
    x8f[:, :, 1:H + 1, 1:W + 1] = x8[:, :, 1:H + 1, 1:W + 1].astype(np.float32)
    r1 = x8f[:, :, 1, :].sum(-1)      # first row sum
    r128 = x8f[:, :, H, :].sum(-1)    # last row sum
    c1 = x8f[:, :, :, 1].sum(-1)      # first col sum
    c128 = x8f[:, :, :, W].sum(-1)    # last col sum
    tot = x8f.sum((2, 3))
    s9 = np.zeros((B, CI, 3, 3), np.float32)  # [b, i, kw, kh] (kw outer!)
    for kh in range(3):
        a = tot.copy()
        if kh == 0:
            a -= r128
        if kh == 2:
            a -= r1
        for kw in range(3):
            v = a.copy()
            if kw == 0:
                d = c128.copy()
                if kh == 0:
                    d -= x8f[:, :, H, W]
                if kh == 2:
                    d -= x8f[:, :, 1, W]
                v -= d
            if kw == 2:
                d = c1.copy()
                if kh == 0:
                    d -= x8f[:, :, H, 1]
                if kh == 2:
                    d -= x8f[:, :, 1, 1]
                v -= d
            s9[:, :, kw, kh] = v
    # device layout [i, b, k] with k = kh*3+kw read as s8[:, b, k]
    # conv tap k=(kh,kw) reads s8[:, b, kw*3+kh]? -> keep [kw, kh] flat:
    # se_chain uses s8[:, b, k] for tap k; map k=(kh,kw) -> idx kw*3+kh.
    s9 = s9 * 0.25

    experts = np.ascontiguousarray(inp["experts"], dtype=np.float32)
    # [E, O, I, K, K] -> [(o_sub 8, e 16)=128, og=16, k=9, i=128]
    ew = experts.reshape(E, 16, 8, CI, 9).transpose(2, 0, 1, 4, 3)
    ew = np.ascontiguousarray(ew).reshape(128, 16, 9, CI)

    idc = np.zeros((128, 129), dtype=ml_dtypes.bfloat16)
    idc[np.arange(128), np.arange(128)] = 1.0
    idc[:, 128] = 1.0

    # banded Toeplitz matrices M[t=(c,dh)][w', w] = tap[c,dh,dw] at
    # w == w' + 3 - dw  (mean channel c=0 scaled by 1/CO)
    saw = np.asarray(inp["sa_w"], np.float32).reshape(2, 7, 7)
    mcm = np.zeros((14, 128, 128), dtype=np.float32)
    for t in range(14):
        c, dh = t // 7, t % 7
        for dw in range(7):
            val = float(saw[c, dh, dw]) * (1.0 / CO if c == 0 else 1.0)
            wp = np.arange(128)
            w = wp + 3 - dw
            m = (w >= 0) & (w < 128)
            mcm[t, wp[m], w[m]] += val
    mc = np.ascontiguousarray(mcm.transpose(1, 0, 2)).astype(ml_dtypes.bfloat16)

    e16t = np.zeros((16, 8, 16), dtype=ml_dtypes.bfloat16)
    for e in range(16):
        e16t[e, :, e] = 1.0
    e16t = e16t.reshape(16, 128)

    # bmask2[p=(osub 8, e 16), col=(j*2+b)] = 1 iff osub == j
    bm = np.zeros((8, 16, 8, BL), dtype=ml_dtypes.float8_e4m3fn)
    for j in range(8):
        bm[j, :, j, :] = 1.0
    bm = bm.reshape(128, 16)

    shared = {
        "ew": (ew * 16.0).astype(ml_dtypes.float8_e4m3fn),
        "idc": idc,
        "mc": mc,
        "rw1t": np.ascontiguousarray(inp["rw1"].T, dtype=np.float32),
        "rw2t": np.ascontiguousarray(inp["rw2"].T, dtype=np.float32),
        "rw3t": np.ascontiguousarray(inp["rw3"].T, dtype=np.float32),
        "caw1t": np.ascontiguousarray(inp["ca_w1"].T, dtype=np.float32),
        "caw2t": np.ascontiguousarray(inp["ca_w2"].T, dtype=np.float32),
        "rbn1_g": np.asarray(inp["rbn1_g"], np.float32),
        "rbn1_b": np.asarray(inp["rbn1_b"], np.float32),
        "rbn2_g": np.asarray(inp["rbn2_g"], np.float32),
        "rbn2_b": np.asarray(inp["rbn2_b"], np.float32),
        "rb3": np.asarray(inp["rb3"], np.float32),
        "ca_bn1_g": np.asarray(inp["ca_bn1_g"], np.float32),
        "ca_bn1_b": np.asarray(inp["ca_bn1_b"], np.float32),
        "ca_bn2_g": np.asarray(inp["ca_bn2_g"], np.float32),
        "ca_bn2_b": np.asarray(inp["ca_bn2_b"], np.float32),
        "sa_bn_g": np.asarray(inp["sa_bn_g"], np.float32),
        "sa_bn_b": np.asarray(inp["sa_bn_b"], np.float32),
        "bmask2": bm,
        "e16t": e16t,
    }
    in_maps = []
    for c in range(NCORES):
        m = dict(shared)
        sl = slice(BL * c, BL * (c + 1))
        m["x8"] = np.ascontiguousarray(x8[sl])
        m["xt"] = np.ascontiguousarray(xt[sl])
        m["pv"] = np.ascontiguousarray(xs[sl].T)  # [i, b]
        # s8 device layout [i, b, 9]; se_chain tap k=(kh,kw) indexes
        # col k -> we must store window sum for (kh,kw) at col kh*3+kw
        s8c = s9[sl].transpose(1, 0, 2, 3)  # [i, b, kw, kh]
        # reorder so flat index k = kh*3+kw: build [i, b, kh, kw] -> flat
        s8c = np.ascontiguousarray(s8c.transpose(0, 1, 3, 2)).reshape(CI, BL, 9)
        m["s8"] = s8c.astype(ml_dtypes.float8_e4m3fn)
        in_maps.append(m)
    return in_maps


def _assemble(results):
    out = np.concatenate([r["out"] for r in results], axis=0)  # [B, W, H, O]
    return np.ascontiguousarray(out.transpose(0, 3, 2, 1)).astype(np.float32)


def get_module():
    if "nc" not in _CACHE:
        _CACHE["nc"] = _build_module()
    return _CACHE["nc"]


def kernel(**inputs):
    nc = get_module()
    in_maps = _host_prep(inputs)
    res = run_bass_kernel_spmd(nc, in_maps, core_ids=list(range(NCORES)))
    return _assemble(res.results)
